# revision 55
# baseline (speedup 1.0000x reference)
"""Trainium2 Bass kernel for nn_Attention (B=4, C=256, L=2048, H=8 heads, D=64).

Sharding: head-parallel across 8 NeuronCores (1 head per core). Each core:
  - projects its head's Q/K/V from the full input x (channels-first, fp16),
  - runs attention in the S^T (keys-on-partitions) layout so softmax's
    denominator comes free from an appended ones-column in the V^T lhsT
    (M=65 matmul),
  - normalizes + casts its head output to fp16,
  - AllToAll redistributes head outputs so each core owns all 8 heads for
    l in [core*256, (core+1)*256) of every batch,
  - each core applies w_out + bias on its column shard.
Host reassembles the 8 column shards into the full [B, C, L] output.

Performance structure (~205-220us vs the 254us starting point; the PE
p-state governor adds +-6us run-to-run variance):
  - the attention S -> exp -> PV chain runs as ONE flat software pipeline
    over all 128 (batch, i-block, key-pair) slots: S+exp for slot s are
    emitted two slots ahead of slot s's PV pair, uniformly across i-block
    and batch boundaries, so the in-order PE FIFO never drains at a block
    restart and never head-of-line blocks on an exp.  Exps are whole-pair
    [128,1024] ops (psA pair tiles, bufs=2 = 4 PSUM banks) on ACT (5
    pairs) / DVE (3 pairs, fused one-pass Schraudolph: fp32 affine
    converted to int16 on write whose bits form the fp16 exponential),
  - the next batch's QKV projection + V^T build and the output
    projections are emitted as small "filler" chunks INSIDE the i-block
    loop (at jp 2/4/6), turning former serial phases into overlap inside
    residual exp-wait windows,
  - the Q/K projection lands as ONE [128,512] PSUM->SBUF copy per
    n-block into qk1=[q|k]; the row-swapped duplicate qk2=[k|q] that the
    two concurrent QK^T quadrants need is made by two SBUF->SBUF DMA
    half-swaps, off the compute engines (GPSIMD and DMA cannot read
    PSUM, so PSUM evacuation stays on DVE/ACT only),
  - ALL output projections run post-loop in collective-COMPLETION order
    (the CC core is FIFO), so nothing inside the main stream ever waits
    on an AllToAll: a waiting gather DMA would head-of-line block the
    in-order Sync queue and stall the bnc writes feeding the final
    collective (CC completion drifts tens of us behind the doorbell when
    the serial CC chain runs late; this cascade was the main source of
    run-to-run tail variance).  Mid-stream PE idle events are poison in
    general: every gap >~0.5us costs a ~3.4us 1.2GHz p-state window,
  - batch 3 processes queries SHARD-STRIDED (each i-block covers 64
    columns of every core's shard) so its AllToAll splits in two
    [8,64,128] pieces: the first overlaps i-blocks 2-3, only the second
    sits on the tail (collective completion latency ~15-20us is fixed
    cost, mostly independent of payload) before a half-shard output
    projection,
  - 12 identity warm-up matmuls engage the PE p-state ramp during the
    initial x DMA; softmax 1/den uses reciprocal_approx_fast on an SBUF
    copy (ACT Reciprocal is blocked for accuracy; exact DVE reciprocal
    costs 3.3us), the yproj bias rides an ACT Identity+bias op.

All matmul operands are fp16 (PSUM accumulation is fp32); measured
end-to-end relative error vs the fp32 reference ~6.2e-3 (tol 2e-2).
"""

import os
import sys
from collections import deque

import numpy as np

sys.path.insert(0, "/opt/trn_rl_repo")

import concourse.bass as bass  # noqa: E402
import concourse.bacc as bacc  # noqa: E402
import concourse.tile as tile  # noqa: E402
import concourse.mybir as mybir  # noqa: E402
import concourse.bass_utils as bass_utils  # noqa: E402
from concourse.bass_interp import get_hw_module  # noqa: E402

B, C, L = 4, 256, 2048
H, D = 8, 64
NCORES = 8
N = B * L                # 8192 flattened (b, l) columns
LSH = L // NCORES        # 256 l-columns per core in the output shard
NBLK = 512               # matmul free-dim block
F32 = mybir.dt.float32
F16 = mybir.dt.float16
I16 = mybir.dt.int16
AF = mybir.ActivationFunctionType
# fp16 Schraudolph exp constants: exp(s) ~= bitcast_f16(int16(s*A + B))
EXP_A = 1024.0 / float(np.log(2.0))
EXP_B = 15.0 * 1024.0 - 44.0

# key-block pairs whose exp runs on the vector engine; the rest run on
# ACT.  3/5 split (DVE also carries the PSUM->SBUF copies, which GpSimd
# cannot: GPSIMD has no PSUM access).
DVE_JPS = frozenset((1, 3, 5))

_CACHE = {}


def _build():
    nc = bacc.Bacc("TRN2", target_bir_lowering=False, debug=False,
                   num_devices=NCORES)

    x_t = nc.dram_tensor("x_t", [2, 128, N], F16, kind="ExternalInput")
    # [c_lo, ch, (q|k) out] merged Q+K projection weights
    wqk_p = nc.dram_tensor("wqk_p", [128, 2, 128], F16, kind="ExternalInput")
    wv_p = nc.dram_tensor("wv_p", [128, 128], F16, kind="ExternalInput")
    wo_p = nc.dram_tensor("wo_p", [128, 4, 256], F16, kind="ExternalInput")
    bias2 = nc.dram_tensor("bias2", [128, 2], F32, kind="ExternalInput")
    out = nc.dram_tensor("out", [B, 2, 128, LSH], F32, kind="ExternalOutput")

    ident_d = nc.inline_tensor(np.eye(64, dtype=np.float16), name="ident64")

    with tile.TileContext(nc) as tc:
        with (
            tc.tile_pool(name="const", bufs=1) as cpool,
            tc.tile_pool(name="qk", bufs=2) as qkpool,
            tc.tile_pool(name="vt", bufs=2) as vtpool,
            tc.tile_pool(name="pt", bufs=6) as ptpool,
            tc.tile_pool(name="small", bufs=2) as spool,
            tc.tile_pool(name="gh", bufs=2) as ghpool,
            tc.tile_pool(name="psA", bufs=2, space="PSUM") as psA,
            tc.tile_pool(name="psO", bufs=2, space="PSUM") as psO,
            tc.tile_pool(name="psP", bufs=2, space="PSUM") as psP,
            tc.tile_pool(name="dram", bufs=1, space="DRAM") as dpool,
        ):
            # ---- weights + first x columns (priority order) ----
            wqk_sb = cpool.tile([128, 256], F16, name="wqk_sb")
            wv_sb = cpool.tile([128, 128], F16, name="wv_sb")
            wo_sb = cpool.tile([128, 1024], F16, name="wo_sb")
            bias_sb = cpool.tile([128, 2], F32, name="bias_sb")
            ident_sb = cpool.tile([64, 64], F16, name="ident_sb")
            nc.sync.dma_start(wqk_sb.rearrange("p (c o) -> p c o", c=2), wqk_p[:])
            nc.sync.dma_start(ident_sb[:], ident_d[:])
            x_sb = cpool.tile([128, 2 * N], F16, name="x_sb")

            def load_x(b, npiece=1):
                step = L // npiece
                for s in range(npiece):
                    for ch in range(2):
                        c0 = ch * N + b * L + s * step
                        nc.sync.dma_start(
                            x_sb[:, c0:c0 + step],
                            x_t[ch, :, b * L + s * step:b * L + (s + 1) * step])

            load_x(0, npiece=4)
            nc.sync.dma_start(wv_sb[:], wv_p[:])
            # warm the PE p-state ramp with throwaway matmuls while x lands
            pswarm = psP.tile([128, 512], F32, name="pswarm", tag="psp")
            for w in range(12):
                nc.tensor.matmul(pswarm[0:64, 0:64], ident_sb[:], ident_sb[:],
                                 start=(w == 0), stop=(w == 11))

            bnc_in = [dpool.tile([NCORES, 64, LSH], F16, name=f"bnc_in{b}",
                                 tag=f"bnc_in{b}")
                      for b in range(B - 1)]
            bnc_out = [dpool.tile([NCORES, 64, LSH], F16, name=f"bnc_out{b}",
                                  tag=f"bnc_out{b}")
                       for b in range(B - 1)]
            # batch 3: two [8, 64, 128] half-collectives (shard cols 0:128 /
            # 128:256), the first overlapping i-blocks 2-3
            bnc3_in = [dpool.tile([NCORES, 64, 128], F16, name=f"bnc3_in{i}",
                                  tag=f"bnc3_in{i}") for i in range(2)]
            bnc3_out = [dpool.tile([NCORES, 64, 128], F16, name=f"bnc3_out{i}",
                                   tag=f"bnc3_out{i}") for i in range(2)]

            qk1 = {}    # [q(0:64) | k(64:128)] x L  -- natural PSUM layout
            qk2 = {}    # [k(0:64) | q(64:128)] x L  -- swapped duplicate
            vt3 = {}
            vc_cur = {}

            def proj_qk_chunk(b, nb):
                """One n-block of batch b's merged Q+K projection."""
                strided = (b == B - 1)
                if nb == 0:
                    qk1[b] = qkpool.tile([128, L], F16, name="qk1", tag="qk1")
                    qk2[b] = qkpool.tile([128, L], F16, name="qk2", tag="qk2")
                ps = psP.tile([128, NBLK], F32, name="psqk", tag="psp")
                for ch in range(2):
                    col0 = ch * N + b * L + nb * NBLK
                    nc.tensor.matmul(
                        ps[:], wqk_sb[:, ch * 128:(ch + 1) * 128],
                        x_sb[:, col0:col0 + NBLK],
                        start=(ch == 0), stop=(ch == 1))
                cols = slice(nb * NBLK, (nb + 1) * NBLK)
                if strided:
                    # batch 3 queries are written shard-interleaved so each
                    # i-block covers 64 columns of every core's shard
                    dq = qk1[b].rearrange("p (i s j) -> p s i j",
                                          i=4, s=8)[0:64, 2 * nb:2 * nb + 2]
                    sq = ps[0:64, :].rearrange("p (s i j) -> p s i j",
                                               s=2, i=4)
                    nc.vector.tensor_copy(dq, sq)
                    nc.vector.tensor_copy(qk1[b][64:128, cols], ps[64:128, :])
                elif b == 0:
                    # startup: build both layouts straight from PSUM, split
                    # across DVE and the otherwise-idle ACT engine
                    if nb % 2 == 0:
                        nc.vector.tensor_copy(qk1[b][:, cols], ps[:])
                        nc.scalar.copy(qk2[b][0:64, cols], ps[64:128, :])
                        nc.scalar.copy(qk2[b][64:128, cols], ps[0:64, :])
                    else:
                        nc.scalar.copy(qk1[b][:, cols], ps[:])
                        nc.vector.tensor_copy(qk2[b][0:64, cols], ps[64:128, :])
                        nc.vector.tensor_copy(qk2[b][64:128, cols], ps[0:64, :])
                else:
                    # one full-height copy covers q and k together
                    nc.vector.tensor_copy(qk1[b][:, cols], ps[:])
                if nb == 3 and b > 0:
                    # swapped duplicate via SBUF->SBUF DMA (off the engines)
                    nc.sync.dma_start(qk2[b][0:64, :], qk1[b][64:128, :])
                    nc.sync.dma_start(qk2[b][64:128, :], qk1[b][0:64, :])

            def proj_v_chunk(b, nbp):
                """V projection, col-strip packed pair of n-blocks."""
                if nbp == 0:
                    vc_cur[b] = vtpool.tile([64, L], F16, name="vc", tag="vc")
                    vt3[b] = vtpool.tile([128, 16 * 65], F16, name="vt", tag="vt"
                                         ).rearrange("p (j e) -> p j e", e=65)
                    nc.vector.memset(vt3[b][:, :, 64], 1.0)
                vc = vc_cur[b]
                psv = psP.tile([128, NBLK], F32, name="psv", tag="psp")
                for strip, nb in ((0, 2 * nbp), (64, 2 * nbp + 1)):
                    o_ap = psv[strip:strip + 64, :]
                    for ch in range(2):
                        col0 = ch * N + b * L + nb * NBLK
                        nc.tensor.matmul(
                            o_ap, wv_sb[:, ch * 64:(ch + 1) * 64],
                            x_sb[:, col0:col0 + NBLK],
                            start=(ch == 0), stop=(ch == 1),
                            tile_position=(0, strip))
                nc.vector.tensor_copy(
                    vc[:, (2 * nbp) * NBLK:(2 * nbp + 1) * NBLK], psv[0:64, :])
                nc.vector.tensor_copy(
                    vc[:, (2 * nbp + 1) * NBLK:(2 * nbp + 2) * NBLK],
                    psv[64:128, :])

            def proj_vt_chunk(b, jg):
                """V^T via PE transpose, two 128-col groups per chunk."""
                vc = vc_cur[b]
                for jt in (2 * jg, 2 * jg + 1):
                    pst = psP.tile([128, 128], F16, name="pst", tag="psp")
                    nc.tensor.transpose(
                        pst[:, 0:64],
                        vc[:, (2 * jt) * 128:(2 * jt + 1) * 128], ident_sb[:])
                    nc.tensor.transpose(
                        pst[:, 64:128],
                        vc[:, (2 * jt + 1) * 128:(2 * jt + 2) * 128],
                        ident_sb[:])
                    nc.vector.tensor_copy(
                        vt3[b][:, 2 * jt:2 * jt + 2, 0:64],
                        pst.rearrange("p (j e) -> p j e", e=64))

            def proj_fillers(b):
                f = []
                for nb in range(4):
                    f.append(lambda b=b, nb=nb: proj_qk_chunk(b, nb))
                for nbp in range(2):
                    f.append(lambda b=b, nbp=nbp: proj_v_chunk(b, nbp))
                for jg in range(4):
                    f.append(lambda b=b, jg=jg: proj_vt_chunk(b, jg))
                return f

            def yproj_oh(b, oh, gh):
                psy = psP.tile([128, LSH], F32, name="psy", tag="psp")
                for c in range(4):
                    nc.tensor.matmul(
                        psy[:],
                        wo_sb[:, c * 256 + oh * 128:c * 256 + (oh + 1) * 128],
                        gh[:, c * LSH:(c + 1) * LSH],
                        start=(c == 0), stop=(c == 3))
                y = spool.tile([128, LSH], F32, name="y", tag="y")
                nc.scalar.add(y[:], psy[:], bias_sb[:, oh:oh + 1])
                nc.sync.dma_start(out[b, oh, :, :], y[:])

            def yproj_fillers(b):
                """Gather + output projection for batch b (a2a(b) complete)."""
                state = {}

                def gather(b=b):
                    gh = ghpool.tile([128, 4 * LSH], F16, name="gh", tag="gh")
                    bo4 = bnc_out[b].rearrange("(hc hp) p l -> hp p hc l", hp=2)
                    for hp in range(2):
                        nc.sync.dma_start(
                            gh[hp * 64:(hp + 1) * 64, :].rearrange(
                                "p (hc l) -> p hc l", hc=4),
                            bo4[hp])
                    state["gh"] = gh

                def oh0(b=b):
                    gather(b)
                    yproj_oh(b, 0, state["gh"])

                def oh1(b=b):
                    yproj_oh(b, 1, state["gh"])
                return [oh0, oh1]

            # ---- flat attention pipeline over all 128 (batch, iblock, jp)
            # pair-slots.  S+exp for slot s are emitted two slots ahead of
            # slot s's PV pair, uniformly across i-block and batch
            # boundaries, so the PE FIFO never drains at a block restart and
            # every exp is a single [128,1024] op (half the per-op overhead
            # of per-half exps; psA pair tiles, bufs=2 = same 4 PSUM banks).
            pts = {}
            pso_of = {}

            def den_chain(b, ib, pso):
                den = spool.tile([1, NBLK], F32, name="den", tag="den")
                nc.scalar.copy(den[:], pso[64:65, :])
                recip = spool.tile([1, NBLK], F32, name="recip", tag="recip")
                nc.vector.reciprocal_approx_fast(recip[:], den[:])
                bc = spool.tile([64, NBLK], F32, name="bc", tag="bc")
                nc.gpsimd.partition_broadcast(bc[:], recip[:])
                on = spool.tile([64, NBLK], F16, name="on", tag="on")
                nc.vector.tensor_mul(on[:], pso[0:64, :], bc[:])
                if b == B - 1:
                    lo = (ib % 2) * 64
                    nc.sync.dma_start(
                        bnc3_in[ib // 2][:, :, lo:lo + 64].rearrange(
                            "s p l -> p s l"),
                        on.rearrange("p (s l) -> p s l", s=8))
                    if ib == 1:
                        emit_a2a3(0)
                else:
                    # both destination l-shards in one DMA
                    nc.sync.dma_start(
                        bnc_in[b][2 * ib:2 * ib + 2, :, :].rearrange(
                            "s p l -> p s l"),
                        on.rearrange("p (s l) -> p s l", s=2))
                    if ib == 3:
                        emit_a2a(b)

            def s_exp(s, filler_q):
                b, ib, jp = s // 32, (s // 8) % 4, s % 8
                if ib == 0 and jp == 0:
                    batch_hooks(b, filler_q)
                qcols = slice(ib * NBLK, (ib + 1) * NBLK)
                jA, jB = 2 * jp, 2 * jp + 1
                # QK^T row-tiled pair: quadrant 1 reads k from qk2-lower / q
                # from qk1-lower, quadrant 2 k from qk1-upper / q qk2-upper
                pss = psA.tile([128, 2 * NBLK], F32, name="pss", tag="pss")
                nc.tensor.matmul(
                    pss[:, 0:NBLK], qk2[b][0:64, jA * 128:(jA + 1) * 128],
                    qk1[b][0:64, qcols],
                    start=True, stop=True, tile_position=(0, 0))
                nc.tensor.matmul(
                    pss[:, NBLK:2 * NBLK],
                    qk1[b][64:128, jB * 128:(jB + 1) * 128],
                    qk2[b][64:128, qcols],
                    start=True, stop=True, tile_position=(64, 0))
                pt_t = ptpool.tile([128, 2 * NBLK], F16, name="pt", tag="pt")
                if jp in DVE_JPS:
                    # fused fp16 Schraudolph exp: one DVE pass, fp32 affine
                    # converted to int16 on write whose bits form the fp16
                    # exponential
                    nc.vector.tensor_scalar(
                        pt_t.bitcast(I16)[:], pss[:], EXP_A, EXP_B,
                        mybir.AluOpType.mult, mybir.AluOpType.add)
                else:
                    nc.scalar.activation(pt_t[:], pss[:], AF.Exp)
                pts[s] = pt_t

            def pv(s):
                b, ib, jp = s // 32, (s // 8) % 4, s % 8
                blk = s // 8
                if jp == 0:
                    pso_of[blk] = psO.tile([65, NBLK], F32, name="pso",
                                           tag="pso")
                pso = pso_of[blk]
                pt_t = pts.pop(s)
                nc.tensor.matmul(pso[:], vt3[b][:, 2 * jp, :],
                                 pt_t[:, 0:NBLK],
                                 start=(jp == 0), stop=False)
                nc.tensor.matmul(pso[:], vt3[b][:, 2 * jp + 1, :],
                                 pt_t[:, NBLK:2 * NBLK],
                                 start=False, stop=(jp == 7))
                if jp == 7:
                    den_chain(b, ib, pso_of.pop(blk))

            def emit_a2a(b):
                nc.gpsimd.collective_compute(
                    "AllToAll", mybir.AluOpType.bypass,
                    replica_groups=[list(range(NCORES))],
                    ins=[bnc_in[b].opt()], outs=[bnc_out[b].opt()])

            def emit_a2a3(half):
                nc.gpsimd.collective_compute(
                    "AllToAll", mybir.AluOpType.bypass,
                    replica_groups=[list(range(NCORES))],
                    ins=[bnc3_in[half].opt()], outs=[bnc3_out[half].opt()])

            def emit_yproj3(half):
                """Output projection for batch 3, half a shard at a time."""
                gh = ghpool.tile([128, 4 * 128], F16, name="gh3", tag="gh")
                bo4 = bnc3_out[half].rearrange("(hc hp) p l -> hp p hc l", hp=2)
                for hp in range(2):
                    nc.sync.dma_start(
                        gh[hp * 64:(hp + 1) * 64, :].rearrange(
                            "p (hc l) -> p hc l", hc=4),
                        bo4[hp])
                # both oh halves accumulate into one y tile so the final
                # store is a single DMA (one less Sync-queue issue on the
                # critical tail path)
                y = spool.tile([128, 256], F32, name="y3", tag="y")
                for oh in range(2):
                    psy = psP.tile([128, 128], F32, name="psy3", tag="psp")
                    for c in range(4):
                        nc.tensor.matmul(
                            psy[:],
                            wo_sb[:, c * 256 + oh * 128:c * 256 + (oh + 1) * 128],
                            gh[:, c * 128:(c + 1) * 128],
                            start=(c == 0), stop=(c == 3))
                    nc.scalar.add(y[:, oh * 128:(oh + 1) * 128], psy[:],
                                  bias_sb[:, oh:oh + 1])
                nc.sync.dma_start(
                    out[B - 1, :, :, half * 128:(half + 1) * 128].rearrange(
                        "o p l -> p o l"),
                    y.rearrange("p (o l) -> p o l", o=2))

            # ---- batch 0 projection on the startup path (qk1/qk2 written
            # per-block across DVE+ACT inside proj_qk_chunk); V-proj chunks
            # and extra warm-up matmuls interleave so the PE never idles on
            # a copy round-trip (idle events reset the p-state ramp) ----
            def warm(n):
                for _ in range(n):
                    nc.tensor.matmul(pswarm[0:64, 0:64], ident_sb[:],
                                     ident_sb[:], start=True, stop=True)

            proj_qk_chunk(0, 0)
            proj_qk_chunk(0, 1)
            warm(2)
            proj_v_chunk(0, 0)
            proj_qk_chunk(0, 2)
            warm(2)
            proj_qk_chunk(0, 3)
            proj_v_chunk(0, 1)
            warm(2)
            for jg in range(4):
                proj_vt_chunk(0, jg)
                if jg < 3:
                    warm(2)

            def batch_hooks(b, filler_q):
                if b + 1 < B:
                    load_x(b + 1)       # stream next batch's x during attention
                if b == 0:
                    nc.sync.dma_start(
                        wo_sb.rearrange("p (c o) -> p c o", c=4), wo_p[:])
                    nc.sync.dma_start(bias_sb[:], bias2[:])
                if b + 1 < B:
                    filler_q.extend(proj_fillers(b + 1))

            filler_q = deque()
            s_exp(0, filler_q)
            s_exp(1, filler_q)
            for s in range(128):
                if s + 2 < 128:
                    s_exp(s + 2, filler_q)
                if s >= 2:
                    pv(s - 2)
                # four filler slots per block spread the fillers' DVE/ACT
                # copies thinly across the whole batch: bunched copies delay
                # pair-exps past the psA recycle slack and stall S pairs
                if s % 2 == 1 and filler_q:
                    filler_q.popleft()()
            pv(126)
            pv(127)
            assert not filler_q, f"{len(filler_q)} fillers left unscheduled"
            emit_a2a3(1)
            # ALL output projections run post-loop, ordered by collective
            # COMPLETION order (the CC core is FIFO).  Nothing inside the
            # main stream ever waits on a collective, so a late-running CC
            # chain can never head-of-line block the in-order Sync queue
            # and delay the bnc writes feeding the final collective; the
            # yproj matmuls fill the PE during that collective's latency.
            for yb in (0, 1, 2):
                for f in yproj_fillers(yb):
                    f()
            emit_yproj3(0)
            emit_yproj3(1)

    nc.compile()
    nc.m = get_hw_module(nc.m)
    return nc


def _prep_in_maps(x, w_qkv, w_out, b_out):
    scale = float(D) ** -0.5
    x = np.asarray(x, np.float32)
    w_qkv = np.asarray(w_qkv, np.float32)
    w_out = np.asarray(w_out, np.float32)
    b_out = np.asarray(b_out, np.float32)

    x_in = np.ascontiguousarray(
        x.transpose(1, 0, 2).reshape(C, N).reshape(2, 128, N)).astype(np.float16)
    wq = w_qkv[0:512].reshape(H, D, C) * scale
    wk = w_qkv[512:1024].reshape(H, D, C)
    wv = w_qkv[1024:1536].reshape(H, D, C)

    wo_p = np.ascontiguousarray(
        w_out.T.reshape(4, 2, 64, 256).transpose(1, 2, 0, 3).reshape(128, 4, 256)
    ).astype(np.float16)
    bias2 = np.ascontiguousarray(b_out.reshape(2, 128).T)

    in_maps = []
    for h in range(NCORES):
        # [c, 128] per half with columns [q 64 | k 64] stacked -> M=128
        wqk = np.concatenate([wq[h].T, wk[h].T], axis=1)  # [256, 128]
        wqk_packed = np.ascontiguousarray(
            wqk.reshape(2, 128, 128).transpose(1, 0, 2)).astype(np.float16)
        wv_packed = np.ascontiguousarray(
            wv[h].T.reshape(2, 128, 64).transpose(1, 0, 2).reshape(128, 128)
        ).astype(np.float16)
        in_maps.append({
            "x_t": x_in,
            "wqk_p": wqk_packed,
            "wv_p": wv_packed,
            "wo_p": wo_p,
            "bias2": bias2,
        })
    return in_maps


def _run(inputs, trace=False):
    if "nc" not in _CACHE:
        _CACHE["nc"] = _build()
    nc = _CACHE["nc"]
    in_maps = _prep_in_maps(**inputs)
    res = bass_utils.run_bass_kernel_spmd(
        nc, in_maps, core_ids=list(range(NCORES)), trace=trace)
    y = np.empty((B, C, L), np.float32)
    for j in range(NCORES):
        shard = res.results[j]["out"].reshape(B, C, LSH)
        y[:, :, j * LSH:(j + 1) * LSH] = shard
    return y, res


def kernel(x, w_qkv, w_out, b_out):
    y, _ = _run(dict(x=x, w_qkv=w_qkv, w_out=w_out, b_out=b_out), trace=False)
    return y


# revision 56
# speedup vs baseline: 1.1083x; 1.1083x over previous
"""Trainium2 Bass kernel for nn_Attention (B=4, C=256, L=2048, H=8 heads, D=64).

Sharding: head-parallel across 8 NeuronCores (1 head per core). Each core:
  - projects its head's Q/K/V from the full input x (channels-first, fp16),
  - runs attention in the S^T (keys-on-partitions) layout so softmax's
    denominator comes free from an appended ones-column in the V^T lhsT
    (M=65 matmul),
  - normalizes + casts its head output to fp16,
  - AllToAll redistributes head outputs so each core owns all 8 heads for
    l in [core*256, (core+1)*256) of every batch,
  - each core applies w_out + bias on its column shard.
Host reassembles the 8 column shards into the full [B, C, L] output.

Performance structure (~205-220us vs the 254us starting point; the PE
p-state governor adds +-6us run-to-run variance):
  - the attention S -> exp -> PV chain runs as ONE flat software pipeline
    over all 128 (batch, i-block, key-pair) slots: S+exp for slot s are
    emitted two slots ahead of slot s's PV pair, uniformly across i-block
    and batch boundaries, so the in-order PE FIFO never drains at a block
    restart and never head-of-line blocks on an exp.  Exps are whole-pair
    [128,1024] ops (psA pair tiles, bufs=2 = 4 PSUM banks) on ACT (5
    pairs) / DVE (3 pairs, fused one-pass Schraudolph: fp32 affine
    converted to int16 on write whose bits form the fp16 exponential),
  - the next batch's QKV projection + V^T build and the output
    projections are emitted as small "filler" chunks INSIDE the i-block
    loop (at jp 2/4/6), turning former serial phases into overlap inside
    residual exp-wait windows,
  - the Q/K projection lands as ONE [128,512] PSUM->SBUF copy per
    n-block into qk1=[q|k]; the row-swapped duplicate qk2=[k|q] that the
    two concurrent QK^T quadrants need is made by two SBUF->SBUF DMA
    half-swaps, off the compute engines (GPSIMD and DMA cannot read
    PSUM, so PSUM evacuation stays on DVE/ACT only),
  - ALL output projections run post-loop in collective-COMPLETION order
    (the CC core is FIFO), so nothing inside the main stream ever waits
    on an AllToAll: a waiting gather DMA would head-of-line block the
    in-order Sync queue and stall the bnc writes feeding the final
    collective (CC completion drifts tens of us behind the doorbell when
    the serial CC chain runs late; this cascade was the main source of
    run-to-run tail variance).  Mid-stream PE idle events are poison in
    general: every gap >~0.5us costs a ~3.4us 1.2GHz p-state window,
  - batch 3 processes queries SHARD-STRIDED (each i-block covers 64
    columns of every core's shard) so its AllToAll splits in two
    [8,64,128] pieces: the first overlaps i-blocks 2-3, only the second
    sits on the tail (collective completion latency ~15-20us is fixed
    cost, mostly independent of payload) before a half-shard output
    projection,
  - 12 identity warm-up matmuls engage the PE p-state ramp during the
    initial x DMA; softmax 1/den uses reciprocal_approx_fast on an SBUF
    copy (ACT Reciprocal is blocked for accuracy; exact DVE reciprocal
    costs 3.3us), the yproj bias rides an ACT Identity+bias op.

All matmul operands are fp16 (PSUM accumulation is fp32); measured
end-to-end relative error vs the fp32 reference ~6.2e-3 (tol 2e-2).
"""

import os
import sys
from collections import deque

import numpy as np

sys.path.insert(0, "/opt/trn_rl_repo")

import concourse.bass as bass  # noqa: E402
import concourse.bacc as bacc  # noqa: E402
import concourse.tile as tile  # noqa: E402
import concourse.mybir as mybir  # noqa: E402
import concourse.bass_utils as bass_utils  # noqa: E402
from concourse.bass_interp import get_hw_module  # noqa: E402

B, C, L = 4, 256, 2048
H, D = 8, 64
NCORES = 8
N = B * L                # 8192 flattened (b, l) columns
LSH = L // NCORES        # 256 l-columns per core in the output shard
NBLK = 512               # matmul free-dim block
F32 = mybir.dt.float32
F16 = mybir.dt.float16
I16 = mybir.dt.int16
AF = mybir.ActivationFunctionType
# fp16 Schraudolph exp constants: exp(s) ~= bitcast_f16(int16(s*A + B))
EXP_A = 1024.0 / float(np.log(2.0))
EXP_B = 15.0 * 1024.0 - 44.0

# key-block pairs whose exp runs on the vector engine; the rest run on
# ACT.  3/5 split (DVE also carries the PSUM->SBUF copies, which GpSimd
# cannot: GPSIMD has no PSUM access).
DVE_JPS = frozenset((1, 3, 5))

_CACHE = {}


def _build():
    nc = bacc.Bacc("TRN2", target_bir_lowering=False, debug=False,
                   num_devices=NCORES)

    x_t = nc.dram_tensor("x_t", [2, 128, N], F16, kind="ExternalInput")
    # [c_lo, ch, (q|k) out] merged Q+K projection weights
    wqk_p = nc.dram_tensor("wqk_p", [128, 2, 128], F16, kind="ExternalInput")
    wv_p = nc.dram_tensor("wv_p", [128, 128], F16, kind="ExternalInput")
    wo_p = nc.dram_tensor("wo_p", [128, 4, 256], F16, kind="ExternalInput")
    bias2 = nc.dram_tensor("bias2", [128, 2], F32, kind="ExternalInput")
    out = nc.dram_tensor("out", [B, 2, 128, LSH], F32, kind="ExternalOutput")

    ident_d = nc.inline_tensor(np.eye(64, dtype=np.float16), name="ident64")

    with tile.TileContext(nc) as tc:
        with (
            tc.tile_pool(name="const", bufs=1) as cpool,
            tc.tile_pool(name="qk", bufs=2) as qkpool,
            tc.tile_pool(name="vt", bufs=2) as vtpool,
            tc.tile_pool(name="pt", bufs=6) as ptpool,
            tc.tile_pool(name="small", bufs=2) as spool,
            tc.tile_pool(name="gh", bufs=2) as ghpool,
            tc.tile_pool(name="psA", bufs=2, space="PSUM") as psA,
            tc.tile_pool(name="psO", bufs=2, space="PSUM") as psO,
            tc.tile_pool(name="psP", bufs=2, space="PSUM") as psP,
            tc.tile_pool(name="dram", bufs=1, space="DRAM") as dpool,
        ):
            # ---- weights + first x columns (priority order) ----
            wqk_sb = cpool.tile([128, 256], F16, name="wqk_sb")
            wv_sb = cpool.tile([128, 128], F16, name="wv_sb")
            wo_sb = cpool.tile([128, 1024], F16, name="wo_sb")
            bias_sb = cpool.tile([128, 2], F32, name="bias_sb")
            ident_sb = cpool.tile([64, 64], F16, name="ident_sb")
            nc.sync.dma_start(wqk_sb.rearrange("p (c o) -> p c o", c=2), wqk_p[:])
            nc.sync.dma_start(ident_sb[:], ident_d[:])
            x_sb = cpool.tile([128, 2 * N], F16, name="x_sb")

            def load_x(b, npiece=1):
                step = L // npiece
                for s in range(npiece):
                    for ch in range(2):
                        c0 = ch * N + b * L + s * step
                        nc.sync.dma_start(
                            x_sb[:, c0:c0 + step],
                            x_t[ch, :, b * L + s * step:b * L + (s + 1) * step])

            load_x(0, npiece=4)
            nc.sync.dma_start(wv_sb[:], wv_p[:])
            # warm the PE p-state ramp with throwaway matmuls while x lands
            pswarm = psP.tile([128, 512], F32, name="pswarm", tag="psp")
            for w in range(12):
                nc.tensor.matmul(pswarm[0:64, 0:64], ident_sb[:], ident_sb[:],
                                 start=(w == 0), stop=(w == 11))

            bnc_in = [dpool.tile([NCORES, 64, LSH], F16, name=f"bnc_in{b}",
                                 tag=f"bnc_in{b}")
                      for b in range(B - 1)]
            bnc_out = [dpool.tile([NCORES, 64, LSH], F16, name=f"bnc_out{b}",
                                  tag=f"bnc_out{b}")
                       for b in range(B - 1)]
            # batch 3: two [8, 64, 128] half-collectives (shard cols 0:128 /
            # 128:256), the first overlapping i-blocks 2-3
            bnc3_in = [dpool.tile([NCORES, 64, 128], F16, name=f"bnc3_in{i}",
                                  tag=f"bnc3_in{i}") for i in range(2)]
            bnc3_out = [dpool.tile([NCORES, 64, 128], F16, name=f"bnc3_out{i}",
                                   tag=f"bnc3_out{i}") for i in range(2)]

            qk1 = {}    # [q(0:64) | k(64:128)] x L  -- natural PSUM layout
            qk2 = {}    # [k(0:64) | q(64:128)] x L  -- swapped duplicate
            vt3 = {}
            vc_cur = {}

            def proj_qk_chunk(b, nb):
                """One n-block of batch b's merged Q+K projection."""
                strided = (b == B - 1)
                if nb == 0:
                    qk1[b] = qkpool.tile([128, L], F16, name="qk1", tag="qk1")
                    qk2[b] = qkpool.tile([128, L], F16, name="qk2", tag="qk2")
                ps = psP.tile([128, NBLK], F32, name="psqk", tag="psp")
                for ch in range(2):
                    col0 = ch * N + b * L + nb * NBLK
                    nc.tensor.matmul(
                        ps[:], wqk_sb[:, ch * 128:(ch + 1) * 128],
                        x_sb[:, col0:col0 + NBLK],
                        start=(ch == 0), stop=(ch == 1))
                cols = slice(nb * NBLK, (nb + 1) * NBLK)
                if strided:
                    # batch 3 queries are written shard-interleaved so each
                    # i-block covers 64 columns of every core's shard
                    dq = qk1[b].rearrange("p (i s j) -> p s i j",
                                          i=4, s=8)[0:64, 2 * nb:2 * nb + 2]
                    sq = ps[0:64, :].rearrange("p (s i j) -> p s i j",
                                               s=2, i=4)
                    nc.vector.tensor_copy(dq, sq)
                    nc.vector.tensor_copy(qk1[b][64:128, cols], ps[64:128, :])
                elif b == 0:
                    # startup: build both layouts straight from PSUM, split
                    # across DVE and the otherwise-idle ACT engine
                    if nb % 2 == 0:
                        nc.vector.tensor_copy(qk1[b][:, cols], ps[:])
                        nc.scalar.copy(qk2[b][0:64, cols], ps[64:128, :])
                        nc.scalar.copy(qk2[b][64:128, cols], ps[0:64, :])
                    else:
                        nc.scalar.copy(qk1[b][:, cols], ps[:])
                        nc.vector.tensor_copy(qk2[b][0:64, cols], ps[64:128, :])
                        nc.vector.tensor_copy(qk2[b][64:128, cols], ps[0:64, :])
                else:
                    # one full-height copy covers q and k together
                    nc.vector.tensor_copy(qk1[b][:, cols], ps[:])
                if nb == 3 and b > 0:
                    # swapped duplicate via SBUF->SBUF DMA (off the engines)
                    nc.sync.dma_start(qk2[b][0:64, :], qk1[b][64:128, :])
                    nc.sync.dma_start(qk2[b][64:128, :], qk1[b][0:64, :])

            def proj_v_chunk(b, nbp):
                """V projection, col-strip packed pair of n-blocks."""
                if nbp == 0:
                    vc_cur[b] = vtpool.tile([64, L], F16, name="vc", tag="vc")
                    vt3[b] = vtpool.tile([128, 16 * 65], F16, name="vt", tag="vt"
                                         ).rearrange("p (j e) -> p j e", e=65)
                    nc.vector.memset(vt3[b][:, :, 64], 1.0)
                vc = vc_cur[b]
                psv = psP.tile([128, NBLK], F32, name="psv", tag="psp")
                for strip, nb in ((0, 2 * nbp), (64, 2 * nbp + 1)):
                    o_ap = psv[strip:strip + 64, :]
                    for ch in range(2):
                        col0 = ch * N + b * L + nb * NBLK
                        nc.tensor.matmul(
                            o_ap, wv_sb[:, ch * 64:(ch + 1) * 64],
                            x_sb[:, col0:col0 + NBLK],
                            start=(ch == 0), stop=(ch == 1),
                            tile_position=(0, strip))
                nc.vector.tensor_copy(
                    vc[:, (2 * nbp) * NBLK:(2 * nbp + 1) * NBLK], psv[0:64, :])
                nc.vector.tensor_copy(
                    vc[:, (2 * nbp + 1) * NBLK:(2 * nbp + 2) * NBLK],
                    psv[64:128, :])

            def proj_vt_chunk(b, jg):
                """V^T via PE transpose, two 128-col groups per chunk."""
                vc = vc_cur[b]
                for jt in (2 * jg, 2 * jg + 1):
                    pst = psP.tile([128, 128], F16, name="pst", tag="psp")
                    nc.tensor.transpose(
                        pst[:, 0:64],
                        vc[:, (2 * jt) * 128:(2 * jt + 1) * 128], ident_sb[:])
                    nc.tensor.transpose(
                        pst[:, 64:128],
                        vc[:, (2 * jt + 1) * 128:(2 * jt + 2) * 128],
                        ident_sb[:])
                    nc.vector.tensor_copy(
                        vt3[b][:, 2 * jt:2 * jt + 2, 0:64],
                        pst.rearrange("p (j e) -> p j e", e=64))

            def proj_fillers(b):
                f = []
                for nb in range(4):
                    f.append(lambda b=b, nb=nb: proj_qk_chunk(b, nb))
                for nbp in range(2):
                    f.append(lambda b=b, nbp=nbp: proj_v_chunk(b, nbp))
                for jg in range(4):
                    f.append(lambda b=b, jg=jg: proj_vt_chunk(b, jg))
                return f

            def yproj_oh(b, oh, gh):
                psy = psP.tile([128, LSH], F32, name="psy", tag="psp")
                for c in range(4):
                    nc.tensor.matmul(
                        psy[:],
                        wo_sb[:, c * 256 + oh * 128:c * 256 + (oh + 1) * 128],
                        gh[:, c * LSH:(c + 1) * LSH],
                        start=(c == 0), stop=(c == 3))
                y = spool.tile([128, LSH], F32, name="y", tag="y")
                nc.scalar.add(y[:], psy[:], bias_sb[:, oh:oh + 1])
                nc.sync.dma_start(out[b, oh, :, :], y[:])

            def yproj_fillers(b):
                """Gather + output projection for batch b (a2a(b) complete)."""
                state = {}

                def gather(b=b):
                    gh = ghpool.tile([128, 4 * LSH], F16, name="gh", tag="gh")
                    bo4 = bnc_out[b].rearrange("(hc hp) p l -> hp p hc l", hp=2)
                    for hp in range(2):
                        nc.sync.dma_start(
                            gh[hp * 64:(hp + 1) * 64, :].rearrange(
                                "p (hc l) -> p hc l", hc=4),
                            bo4[hp])
                    state["gh"] = gh

                def oh0(b=b):
                    gather(b)
                    yproj_oh(b, 0, state["gh"])

                def oh1(b=b):
                    yproj_oh(b, 1, state["gh"])
                return [oh0, oh1]

            # ---- flat attention pipeline over all 128 (batch, iblock, jp)
            # pair-slots.  S+exp for slot s are emitted two slots ahead of
            # slot s's PV pair, uniformly across i-block and batch
            # boundaries, so the PE FIFO never drains at a block restart and
            # every exp is a single [128,1024] op (half the per-op overhead
            # of per-half exps; psA pair tiles, bufs=2 = same 4 PSUM banks).
            pts = {}
            pso_of = {}

            def den_chain(b, ib, pso):
                den = spool.tile([1, NBLK], F32, name="den", tag="den")
                nc.scalar.copy(den[:], pso[64:65, :])
                recip = spool.tile([1, NBLK], F32, name="recip", tag="recip")
                nc.vector.reciprocal_approx_fast(recip[:], den[:])
                bc = spool.tile([64, NBLK], F32, name="bc", tag="bc")
                nc.gpsimd.partition_broadcast(bc[:], recip[:])
                on = spool.tile([64, NBLK], F16, name="on", tag="on")
                nc.vector.tensor_mul(on[:], pso[0:64, :], bc[:])
                if b == B - 1:
                    lo = (ib % 2) * 64
                    nc.sync.dma_start(
                        bnc3_in[ib // 2][:, :, lo:lo + 64].rearrange(
                            "s p l -> p s l"),
                        on.rearrange("p (s l) -> p s l", s=8))
                    if ib == 1:
                        emit_a2a3(0)
                else:
                    # both destination l-shards in one DMA
                    nc.sync.dma_start(
                        bnc_in[b][2 * ib:2 * ib + 2, :, :].rearrange(
                            "s p l -> p s l"),
                        on.rearrange("p (s l) -> p s l", s=2))
                    if ib == 3:
                        emit_a2a(b)

            def s_exp(s, filler_q):
                b, ib, jp = s // 32, (s // 8) % 4, s % 8
                if ib == 0 and jp == 0:
                    batch_hooks(b, filler_q)
                qcols = slice(ib * NBLK, (ib + 1) * NBLK)
                jA, jB = 2 * jp, 2 * jp + 1
                # QK^T row-tiled pair: quadrant 1 reads k from qk2-lower / q
                # from qk1-lower, quadrant 2 k from qk1-upper / q qk2-upper
                pss = psA.tile([128, 2 * NBLK], F32, name="pss", tag="pss")
                nc.tensor.matmul(
                    pss[:, 0:NBLK], qk2[b][0:64, jA * 128:(jA + 1) * 128],
                    qk1[b][0:64, qcols],
                    start=True, stop=True, tile_position=(0, 0))
                nc.tensor.matmul(
                    pss[:, NBLK:2 * NBLK],
                    qk1[b][64:128, jB * 128:(jB + 1) * 128],
                    qk2[b][64:128, qcols],
                    start=True, stop=True, tile_position=(64, 0))
                pt_t = ptpool.tile([128, 2 * NBLK], F16, name="pt", tag="pt")
                if jp in DVE_JPS:
                    # fused fp16 Schraudolph exp: one DVE pass, fp32 affine
                    # converted to int16 on write whose bits form the fp16
                    # exponential
                    nc.vector.tensor_scalar(
                        pt_t.bitcast(I16)[:], pss[:], EXP_A, EXP_B,
                        mybir.AluOpType.mult, mybir.AluOpType.add)
                else:
                    nc.scalar.activation(pt_t[:], pss[:], AF.Exp)
                pts[s] = pt_t

            def pv(s):
                b, ib, jp = s // 32, (s // 8) % 4, s % 8
                blk = s // 8
                if jp == 0:
                    pso_of[blk] = psO.tile([65, NBLK], F32, name="pso",
                                           tag="pso")
                pso = pso_of[blk]
                pt_t = pts.pop(s)
                nc.tensor.matmul(pso[:], vt3[b][:, 2 * jp, :],
                                 pt_t[:, 0:NBLK],
                                 start=(jp == 0), stop=False)
                nc.tensor.matmul(pso[:], vt3[b][:, 2 * jp + 1, :],
                                 pt_t[:, NBLK:2 * NBLK],
                                 start=False, stop=(jp == 7))
                if jp == 7:
                    den_chain(b, ib, pso_of.pop(blk))

            def emit_a2a(b):
                nc.gpsimd.collective_compute(
                    "AllToAll", mybir.AluOpType.bypass,
                    replica_groups=[list(range(NCORES))],
                    ins=[bnc_in[b].opt()], outs=[bnc_out[b].opt()])

            def emit_a2a3(half):
                nc.gpsimd.collective_compute(
                    "AllToAll", mybir.AluOpType.bypass,
                    replica_groups=[list(range(NCORES))],
                    ins=[bnc3_in[half].opt()], outs=[bnc3_out[half].opt()])

            def emit_yproj3(half):
                """Output projection for batch 3, half a shard at a time."""
                gh = ghpool.tile([128, 4 * 128], F16, name="gh3", tag="gh")
                bo4 = bnc3_out[half].rearrange("(hc hp) p l -> hp p hc l", hp=2)
                for hp in range(2):
                    nc.sync.dma_start(
                        gh[hp * 64:(hp + 1) * 64, :].rearrange(
                            "p (hc l) -> p hc l", hc=4),
                        bo4[hp])
                for oh in range(2):
                    psy = psP.tile([128, 128], F32, name="psy3", tag="psp")
                    for c in range(4):
                        nc.tensor.matmul(
                            psy[:],
                            wo_sb[:, c * 256 + oh * 128:c * 256 + (oh + 1) * 128],
                            gh[:, c * 128:(c + 1) * 128],
                            start=(c == 0), stop=(c == 3))
                    y = spool.tile([128, 128], F32, name="y3", tag="y")
                    nc.scalar.add(y[:], psy[:], bias_sb[:, oh:oh + 1])
                    nc.sync.dma_start(
                        out[B - 1, oh, :, half * 128:(half + 1) * 128], y[:])

            # ---- batch 0 projection on the startup path (qk1/qk2 written
            # per-block across DVE+ACT inside proj_qk_chunk); V-proj chunks
            # and extra warm-up matmuls interleave so the PE never idles on
            # a copy round-trip (idle events reset the p-state ramp) ----
            def warm(n):
                for _ in range(n):
                    nc.tensor.matmul(pswarm[0:64, 0:64], ident_sb[:],
                                     ident_sb[:], start=True, stop=True)

            proj_qk_chunk(0, 0)
            proj_qk_chunk(0, 1)
            warm(2)
            proj_v_chunk(0, 0)
            proj_qk_chunk(0, 2)
            warm(2)
            proj_qk_chunk(0, 3)
            proj_v_chunk(0, 1)
            warm(2)
            for jg in range(4):
                proj_vt_chunk(0, jg)
                if jg < 3:
                    warm(2)

            def batch_hooks(b, filler_q):
                if b + 1 < B:
                    load_x(b + 1)       # stream next batch's x during attention
                if b == 0:
                    nc.sync.dma_start(
                        wo_sb.rearrange("p (c o) -> p c o", c=4), wo_p[:])
                    nc.sync.dma_start(bias_sb[:], bias2[:])
                if b + 1 < B:
                    filler_q.extend(proj_fillers(b + 1))

            filler_q = deque()
            s_exp(0, filler_q)
            s_exp(1, filler_q)
            for s in range(128):
                if s + 2 < 128:
                    s_exp(s + 2, filler_q)
                if s >= 2:
                    pv(s - 2)
                # four filler slots per block spread the fillers' DVE/ACT
                # copies thinly across the whole batch: bunched copies delay
                # pair-exps past the psA recycle slack and stall S pairs
                if s % 2 == 1 and filler_q:
                    filler_q.popleft()()
            pv(126)
            pv(127)
            assert not filler_q, f"{len(filler_q)} fillers left unscheduled"
            emit_a2a3(1)
            # ALL output projections run post-loop, ordered by collective
            # COMPLETION order (the CC core is FIFO).  Nothing inside the
            # main stream ever waits on a collective, so a late-running CC
            # chain can never head-of-line block the in-order Sync queue
            # and delay the bnc writes feeding the final collective; the
            # yproj matmuls fill the PE during that collective's latency.
            for yb in (0, 1, 2):
                for f in yproj_fillers(yb):
                    f()
            emit_yproj3(0)
            emit_yproj3(1)

    nc.compile()
    nc.m = get_hw_module(nc.m)
    return nc


def _prep_in_maps(x, w_qkv, w_out, b_out):
    scale = float(D) ** -0.5
    x = np.asarray(x, np.float32)
    w_qkv = np.asarray(w_qkv, np.float32)
    w_out = np.asarray(w_out, np.float32)
    b_out = np.asarray(b_out, np.float32)

    x_in = np.ascontiguousarray(
        x.transpose(1, 0, 2).reshape(C, N).reshape(2, 128, N)).astype(np.float16)
    wq = w_qkv[0:512].reshape(H, D, C) * scale
    wk = w_qkv[512:1024].reshape(H, D, C)
    wv = w_qkv[1024:1536].reshape(H, D, C)

    wo_p = np.ascontiguousarray(
        w_out.T.reshape(4, 2, 64, 256).transpose(1, 2, 0, 3).reshape(128, 4, 256)
    ).astype(np.float16)
    bias2 = np.ascontiguousarray(b_out.reshape(2, 128).T)

    in_maps = []
    for h in range(NCORES):
        # [c, 128] per half with columns [q 64 | k 64] stacked -> M=128
        wqk = np.concatenate([wq[h].T, wk[h].T], axis=1)  # [256, 128]
        wqk_packed = np.ascontiguousarray(
            wqk.reshape(2, 128, 128).transpose(1, 0, 2)).astype(np.float16)
        wv_packed = np.ascontiguousarray(
            wv[h].T.reshape(2, 128, 64).transpose(1, 0, 2).reshape(128, 128)
        ).astype(np.float16)
        in_maps.append({
            "x_t": x_in,
            "wqk_p": wqk_packed,
            "wv_p": wv_packed,
            "wo_p": wo_p,
            "bias2": bias2,
        })
    return in_maps


def _run(inputs, trace=False):
    if "nc" not in _CACHE:
        _CACHE["nc"] = _build()
    nc = _CACHE["nc"]
    in_maps = _prep_in_maps(**inputs)
    res = bass_utils.run_bass_kernel_spmd(
        nc, in_maps, core_ids=list(range(NCORES)), trace=trace)
    y = np.empty((B, C, L), np.float32)
    for j in range(NCORES):
        shard = res.results[j]["out"].reshape(B, C, LSH)
        y[:, :, j * LSH:(j + 1) * LSH] = shard
    return y, res


def kernel(x, w_qkv, w_out, b_out):
    y, _ = _run(dict(x=x, w_qkv=w_qkv, w_out=w_out, b_out=b_out), trace=False)
    return y
